# revision 21
# baseline (speedup 1.0000x reference)
"""Trainium2 Bass kernel: custom inverse STFT (degenerate per-bin rotation +
Hann window + overlap-add + window correction).

Math (matching the reference):
    F[i,k]  = S_real[i,k]*A[k] + S_imag[i,k]*B[k]
      A[k]  = w[k]*(cos(th)-sin(th))/n,  B[k] = -w[k]*(cos(th)+sin(th))/n
    out[t]  = sum_i F[i, t-256*i] / max(corr[t], 1e-8)

Implementation (fp16 inputs, bf16 products, f32 accumulation):
  - Inputs are cast to fp16 on the host (halves HBM traffic; the 2e-2 rel-err
    budget dwarfs 16-bit rounding).  Coefficients are scaled by 2^8 so
    products stay in fp16's normal range; the host divides the scale out.
  - Sharding: 8192 frames -> 8 cores x 1024 frames (+3 left-halo frames).
  - Per core: 8 slices of 128 consecutive frames starting at multiples of
    125 (slices overlap by 3 frames), frame = partition.  Each slice yields
    125 output blocks whose 4 overlap-add contributions all live on
    partitions of the SAME slice, so the whole overlap-add runs on the
    TensorEngine as shifted-identity matmuls accumulating in f32 PSUM.
  - Schedule (vs the original version): the coefficient broadcast runs as
    16 back-to-back N=128 matmuls right after the tiny Crow DMA lands (fast
    even at the cold 1.2GHz PE clock) and doubles as the HAM warm-up, the
    A-half staging copies run on the DVE (B-half on ACT), the overlap-add
    is slice-granular (8 N=256 matmuls per slice, issued as soon as that
    slice's product finishes), and the first/last slices are split in
    column halves so the pipeline starts earlier and ends on a small
    64KB final store.
  - The 6 edge blocks and each core's last 24 blocks + global tail are
    recomputed exactly in f32 on the host (corr < 2 there amplifies fp16
    error; also keeps the device graph uniform).
"""

import numpy as np
import ml_dtypes

import concourse.bass as bass
import concourse.bacc as bacc
import concourse.mybir as mybir
import concourse.tile as tile
from concourse.bass_utils import run_bass_kernel_spmd

F16 = mybir.dt.float16
BF16 = mybir.dt.bfloat16
F32 = mybir.dt.float32
ALU = mybir.AluOpType

P = 128            # SBUF partitions
FL = 1024          # frame length (== fft length)
FS = 256           # frame step
NF = 8192          # total frames
NCORES = 8
FPC = NF // NCORES          # frames owned per core (1024)
SL = 125                    # slice stride in frames (overlap of 3)
NS = 8                      # slices x 125 blocks (the last 24 blocks per
                            # core are recomputed on the host, like edges)
W2 = 2 * FL                 # interleaved Sr|Si row width (2048)
OUT_LEN = FS * (NF - 1) + FL
SCALE = np.float32(256.0)   # exact power-of-2 coefficient pre-scale


def _window32():
    # bit-matches the reference's f32 window computation
    k = np.arange(FL, dtype=np.float32)
    th = np.float32(2.0 * np.pi) * k / np.float32(FL)
    return (np.float32(0.5) - np.float32(0.5) * np.cos(th)).astype(np.float32)


def _coeffs32():
    k = np.arange(FL, dtype=np.float64)
    th = 2.0 * np.pi * k / FL
    w = _window32().astype(np.float64)
    a = (w * (np.cos(th) - np.sin(th)) / FL).astype(np.float32)
    b = (-w * (np.cos(th) + np.sin(th)) / FL).astype(np.float32)
    return a, b


def _window_correction():
    w = _window32()
    corr = np.zeros(OUT_LEN, dtype=np.float32)
    for j in range(4):
        view = corr[j * FS:j * FS + NF * FS].reshape(NF, FS)
        view += w[j * FS:(j + 1) * FS][None, :]
    return corr


def build_nc():
    nc = bacc.Bacc(trn_type="TRN2", target_bir_lowering=False, debug=False)
    x_d = nc.dram_tensor("x", [P * NS * W2], F16, kind="ExternalInput").ap()
    cf_d = nc.dram_tensor("coefs", [W2 + P], F16, kind="ExternalInput").ap()
    id_d = nc.dram_tensor("ident", [P, P], BF16, kind="ExternalInput").ap()
    out_d = nc.dram_tensor("out_seg", [P * NS * FS], F16, kind="ExternalOutput").ap()

    xv = x_d.rearrange("(p s k) -> p s k", p=P, s=NS)   # [128, 8, 2048]
    ov = out_d.rearrange("(p x) -> p x", p=P)

    with tile.TileContext(nc) as tc:
        with (
            tc.tile_pool(name="const", bufs=1) as cpool,
            tc.tile_pool(name="main", bufs=1) as mpool,
            tc.tile_pool(name="psumc", bufs=1, space="PSUM") as ppool,
            tc.tile_pool(name="psum", bufs=4, space="PSUM") as qpool,
        ):
            Crow = cpool.tile([1, W2 + P], F16, tag="Crow")  # A*2^8|B*2^8|ones
            Id = cpool.tile([P, P], BF16, tag="Id")
            ABt = cpool.tile([P, W2], F16, tag="ABt")
            X = mpool.tile([P, NS * W2], F16, tag="X")
            T = mpool.tile([P, NS * W2], BF16, tag="T")
            Ot = mpool.tile([P, NS * FS], F16, tag="Ot")

            def xs(s, c0, c1):
                return X[:, s * W2 + c0:s * W2 + c1]

            # --- input stream ---------------------------------------------
            # DMA completion sems lag the last byte by ~2.5us (HBM write
            # receipt under load), so consumption is sem-paced; full 512KB
            # slices keep the queues at line rate, and only the tail slice
            # is split so the final chain after the last sem is short.
            # sync queue:   Crow, X0, X2, X4, X6            (2.05 MB + stores)
            # scalar queue: Id,   X1, X3, X5, X7.k1, X7.k2  (2.08 MB + store)
            nc.sync.dma_start(out=Crow[:, :], in_=cf_d[None, :])
            nc.sync.dma_start(out=xs(0, 0, FL), in_=xv[:, 0, 0:FL])
            nc.scalar.dma_start(out=Id[:, :], in_=id_d[:, :])
            nc.scalar.dma_start(out=xs(0, FL, W2), in_=xv[:, 0, FL:W2])
            nc.sync.dma_start(out=xs(1, 0, W2), in_=xv[:, 1, :])
            nc.scalar.dma_start(out=xs(2, 0, W2), in_=xv[:, 2, :])
            nc.sync.dma_start(out=xs(3, 0, W2), in_=xv[:, 3, :])
            nc.scalar.dma_start(out=xs(4, 0, W2), in_=xv[:, 4, :])

            # --- coefficient broadcast ------------------------------------
            # ones @ row as 16 back-to-back N=128 matmuls: ~107ns each even
            # at the cold PE clock, and the burst doubles as HAM warm-up.
            # A and B go to separate PSUM tiles so each staging cast waits
            # only on its own 8 matmuls.
            ones = Crow[0:1, W2:W2 + P]
            ABpA = ppool.tile([P, FL], F32, tag="ABpA")
            ABpB = ppool.tile([P, FL], F32, tag="ABpB")
            for j in range(8):
                nc.tensor.matmul(ABpA[:, j * 128:(j + 1) * 128], ones,
                                 Crow[0:1, j * 128:(j + 1) * 128],
                                 start=True, stop=True)
            nc.vector.tensor_copy(out=ABt[:, 0:FL], in_=ABpA[:, :])
            for j in range(8):
                nc.tensor.matmul(ABpB[:, j * 128:(j + 1) * 128], ones,
                                 Crow[0:1, FL + j * 128:FL + (j + 1) * 128],
                                 start=True, stop=True)
            nc.scalar.copy(out=ABt[:, FL:W2], in_=ABpB[:, :])

            # rest of the input stream (emitted after the casts so the B
            # cast sits before the late gens in the ACT queue's FIFO)
            nc.sync.dma_start(out=xs(5, 0, W2), in_=xv[:, 5, :])
            nc.scalar.dma_start(out=xs(6, 0, W2), in_=xv[:, 6, :])
            nc.sync.dma_start(out=xs(7, 0, FL), in_=xv[:, 7, 0:FL])
            nc.scalar.dma_start(out=xs(7, FL, FL + 768), in_=xv[:, 7, FL:FL + 768])
            nc.sync.dma_start(out=xs(7, FL + 768, W2), in_=xv[:, 7, FL + 768:W2])

            # --- per-slice product + overlap-add --------------------------
            def product(s, c0, c1):
                # T[:, sW2+c0 : sW2+c1] = X ⊙ ABt over that column range
                nc.vector.tensor_tensor(out=T[:, s * W2 + c0:s * W2 + c1],
                                        in0=xs(s, c0, c1),
                                        in1=ABt[:, c0:c1], op=ALU.mult)

            def mms(opt, s, k, cs, start, stop):
                # shifted-identity matmuls for overlap chunks cs of half k
                for c in cs:
                    w = Id[:, 3 - c:3 - c + SL]
                    rhs = T[:, s * W2 + k * FL + c * FS:
                            s * W2 + k * FL + (c + 1) * FS]
                    nc.tensor.matmul(opt[0:SL, 0:FS], w, rhs,
                                     start=(start and c == cs[0]),
                                     stop=(stop and c == cs[-1]))

            for s in range(NS):
                opt = qpool.tile([P, 512], F32, tag="ps", name=f"Opt{s}")
                if s == 0:
                    product(s, 0, FL)
                    mms(opt, s, 0, (0, 1, 2, 3), True, False)
                    product(s, FL, W2)
                    mms(opt, s, 1, (0, 1, 2, 3), False, True)
                elif s == 7:
                    # tail slice in three pieces.  The c=3 chunk of the
                    # overlap-add is partition-aligned (block m reads row
                    # m+3-c = m), so the last piece skips the PE entirely:
                    # after the final DMA sem the chain is one 0.3us DVE
                    # product + one DVE add (PSUM + product -> fp16 out).
                    product(s, 0, FL)
                    mms(opt, s, 0, (0, 1, 2, 3), True, False)
                    product(s, FL, FL + 768)
                    mms(opt, s, 1, (0, 1, 2), False, True)
                    product(s, FL + 768, W2)
                else:
                    product(s, 0, W2)
                    mms(opt, s, 0, (0, 1, 2, 3), True, False)
                    mms(opt, s, 1, (0, 1, 2, 3), False, True)
                # PSUM -> SBUF fp16 staging; the last slice folds the c=3
                # product in during the evacuation add on the DVE
                dst = Ot[0:SL, s * FS:(s + 1) * FS]
                if s == 7:
                    nc.vector.tensor_tensor(
                        out=dst, in0=opt[0:SL, 0:FS],
                        in1=T[0:SL, s * W2 + FL + 768:(s + 1) * W2],
                        op=ALU.add)
                else:
                    nc.scalar.copy(out=dst, in_=opt[0:SL, 0:FS])
                # stores: both mid stores ride the scalar ring AFTER X7.k2 in
                # FIFO order so their bytes never delay the input tail; the
                # small final store goes on the idle sync ring
                if s == 2:
                    nc.scalar.dma_start(out=ov[:, 0:3 * FS], in_=Ot[:, 0:3 * FS])
                elif s == 6:
                    nc.scalar.dma_start(out=ov[:, 3 * FS:7 * FS],
                                        in_=Ot[:, 3 * FS:7 * FS])
                elif s == 7:
                    nc.sync.dma_start(out=ov[:, 7 * FS:8 * FS],
                                      in_=Ot[:, 7 * FS:8 * FS])
    nc.compile()
    return nc


_cache = {}


def _get_nc():
    if "nc" not in _cache:
        _cache["nc"] = build_nc()
    return _cache["nc"]


def make_in_maps(S_real, S_imag):
    a32, b32 = _coeffs32()
    coefs = np.zeros(W2 + P, dtype=np.float16)
    coefs[0:FL] = (a32 * SCALE).astype(np.float16)
    coefs[FL:W2] = (b32 * SCALE).astype(np.float16)
    coefs[W2:] = np.float16(1.0)
    ident = np.eye(P, dtype=ml_dtypes.bfloat16)

    # interleaved + padded fp16 input: row r of core m = global frame
    # m*1024 - 3 + r (zeros outside [0, NF))
    sr16 = S_real.astype(np.float16)
    si16 = S_imag.astype(np.float16)
    glob = np.zeros((3 + NF + P, W2), dtype=np.float16)
    glob[3:3 + NF, 0:FL] = sr16
    glob[3:3 + NF, FL:W2] = si16

    in_maps = []
    for m in range(NCORES):
        base = m * FPC
        x = np.empty((P, NS, W2), dtype=np.float16)
        for s in range(NS):
            x[:, s, :] = glob[base + s * SL:base + s * SL + P]
        in_maps.append({
            "x": x.reshape(-1),
            "coefs": coefs,
            "ident": ident,
        })
    return in_maps


def assemble_output(S_real, S_imag, segs):
    a32, b32 = _coeffs32()
    full = np.empty(OUT_LEN, dtype=np.float32)
    inv_scale = np.float32(1.0) / SCALE
    for m in range(NCORES):
        # seg[p, s*256+r] -> block s*125+p (1000 blocks from the device)
        v = segs[m].reshape(P, NS, FS)[0:SL].transpose(1, 0, 2).reshape(-1)
        bt = m * FPC * FS
        full[bt:bt + NS * SL * FS] = v.astype(np.float32) * inv_scale
        # the core's last 24 blocks: exact f32 on the host
        f0 = m * FPC + 997
        Fr = S_real[f0:f0 + 27] * a32[None, :] + S_imag[f0:f0 + 27] * b32[None, :]
        for lb in range(NS * SL, FPC):
            acc = Fr[lb - 997, 0:FS].copy()
            for c in range(1, 4):
                acc += Fr[lb - c - 997, c * FS:(c + 1) * FS]
            full[(m * FPC + lb) * FS:(m * FPC + lb + 1) * FS] = acc

    # exact f32 recompute of the 6 edge blocks (corr < 2 there: the final
    # division amplifies fp16 error by up to ~1e5) and the global tail
    Fh = S_real[0:3] * a32[None, :] + S_imag[0:3] * b32[None, :]
    full[0:FS] = Fh[0, 0:FS]
    full[FS:2 * FS] = Fh[0, FS:2 * FS] + Fh[1, 0:FS]
    full[2 * FS:3 * FS] = Fh[0, 2 * FS:3 * FS] + Fh[1, FS:2 * FS] + Fh[2, 0:FS]
    Ft = S_real[NF - 3:] * a32[None, :] + S_imag[NF - 3:] * b32[None, :]
    full[NF * FS:NF * FS + FS] = Ft[0, 3 * FS:] + Ft[1, 2 * FS:3 * FS] + Ft[2, FS:2 * FS]
    full[NF * FS + FS:NF * FS + 2 * FS] = Ft[1, 3 * FS:] + Ft[2, 2 * FS:3 * FS]
    full[NF * FS + 2 * FS:] = Ft[2, 3 * FS:]

    if "corr" not in _cache:
        _cache["corr"] = np.maximum(_window_correction(), np.float32(1e-8))
    return full / _cache["corr"]


def kernel(S_real, S_imag):
    S_real = np.asarray(S_real, dtype=np.float32)
    S_imag = np.asarray(S_imag, dtype=np.float32)
    in_maps = make_in_maps(S_real, S_imag)
    nc = _get_nc()
    res = run_bass_kernel_spmd(nc, in_maps, list(range(NCORES)))
    segs = [res.results[m]["out_seg"] for m in range(NCORES)]
    return assemble_output(S_real, S_imag, segs)


# revision 26
# speedup vs baseline: 1.0117x; 1.0117x over previous
"""Trainium2 Bass kernel: custom inverse STFT (degenerate per-bin rotation +
Hann window + overlap-add + window correction).

Math (matching the reference):
    F[i,k]  = S_real[i,k]*A[k] + S_imag[i,k]*B[k]
      A[k]  = w[k]*(cos(th)-sin(th))/n,  B[k] = -w[k]*(cos(th)+sin(th))/n
    out[t]  = sum_i F[i, t-256*i] / max(corr[t], 1e-8)

Implementation (fp16 inputs, bf16 products, f32 accumulation):
  - Inputs are cast to fp16 on the host (halves HBM traffic; the 2e-2 rel-err
    budget dwarfs 16-bit rounding).  Coefficients are scaled by 2^8 so
    products stay in fp16's normal range; the host divides the scale out.
  - Sharding: 8192 frames -> 8 cores x 1024 frames (+3 left-halo frames).
  - Per core: 8 slices of 128 consecutive frames starting at multiples of
    125 (slices overlap by 3 frames), frame = partition.  Each slice yields
    125 output blocks whose 4 overlap-add contributions all live on
    partitions of the SAME slice, so the whole overlap-add runs on the
    TensorEngine as shifted-identity matmuls accumulating in f32 PSUM.
  - Schedule (vs the original version): the coefficient broadcast runs as
    16 back-to-back N=128 matmuls right after the tiny Crow DMA lands (fast
    even at the cold 1.2GHz PE clock) and doubles as the HAM warm-up, the
    A-half staging copies run on the DVE (B-half on ACT), the overlap-add
    is slice-granular (8 N=256 matmuls per slice, issued as soon as that
    slice's product finishes), and the first/last slices are split in
    column halves so the pipeline starts earlier and ends on a small
    64KB final store.
  - The 6 edge blocks and each core's last 24 blocks + global tail are
    recomputed exactly in f32 on the host (corr < 2 there amplifies fp16
    error; also keeps the device graph uniform).
"""

import numpy as np
import ml_dtypes

import concourse.bass as bass
import concourse.bacc as bacc
import concourse.mybir as mybir
import concourse.tile as tile
from concourse.bass_utils import run_bass_kernel_spmd

F16 = mybir.dt.float16
BF16 = mybir.dt.bfloat16
F32 = mybir.dt.float32
ALU = mybir.AluOpType

P = 128            # SBUF partitions
FL = 1024          # frame length (== fft length)
FS = 256           # frame step
NF = 8192          # total frames
NCORES = 8
FPC = NF // NCORES          # frames owned per core (1024)
SL = 125                    # slice stride in frames (overlap of 3)
NS = 8                      # slices x 125 blocks (the last 24 blocks per
                            # core are recomputed on the host, like edges)
W2 = 2 * FL                 # interleaved Sr|Si row width (2048)
OUT_LEN = FS * (NF - 1) + FL
SCALE = np.float32(256.0)   # exact power-of-2 coefficient pre-scale


def _window32():
    # bit-matches the reference's f32 window computation
    k = np.arange(FL, dtype=np.float32)
    th = np.float32(2.0 * np.pi) * k / np.float32(FL)
    return (np.float32(0.5) - np.float32(0.5) * np.cos(th)).astype(np.float32)


def _coeffs32():
    k = np.arange(FL, dtype=np.float64)
    th = 2.0 * np.pi * k / FL
    w = _window32().astype(np.float64)
    a = (w * (np.cos(th) - np.sin(th)) / FL).astype(np.float32)
    b = (-w * (np.cos(th) + np.sin(th)) / FL).astype(np.float32)
    return a, b


def _window_correction():
    w = _window32()
    corr = np.zeros(OUT_LEN, dtype=np.float32)
    for j in range(4):
        view = corr[j * FS:j * FS + NF * FS].reshape(NF, FS)
        view += w[j * FS:(j + 1) * FS][None, :]
    return corr


def build_nc():
    nc = bacc.Bacc(trn_type="TRN2", target_bir_lowering=False, debug=False)
    x_d = nc.dram_tensor("x", [P * NS * W2], F16, kind="ExternalInput").ap()
    cf_d = nc.dram_tensor("coefs", [W2 + P], F16, kind="ExternalInput").ap()
    id_d = nc.dram_tensor("ident", [P, P], BF16, kind="ExternalInput").ap()
    out_d = nc.dram_tensor("out_seg", [P * NS * FS], F16, kind="ExternalOutput").ap()

    xv = x_d.rearrange("(p s k) -> p s k", p=P, s=NS)   # [128, 8, 2048]
    ov = out_d.rearrange("(p x) -> p x", p=P)

    with tile.TileContext(nc) as tc:
        with (
            tc.tile_pool(name="const", bufs=1) as cpool,
            tc.tile_pool(name="main", bufs=1) as mpool,
            tc.tile_pool(name="psumc", bufs=1, space="PSUM") as ppool,
            tc.tile_pool(name="psum", bufs=4, space="PSUM") as qpool,
        ):
            Crow = cpool.tile([1, W2 + P], F16, tag="Crow")  # A*2^8|B*2^8|ones
            Id = cpool.tile([P, P], BF16, tag="Id")
            ABt = cpool.tile([P, W2], F16, tag="ABt")
            X = mpool.tile([P, NS * W2], F16, tag="X")
            T = mpool.tile([P, NS * W2], BF16, tag="T")
            Ot = mpool.tile([P, NS * FS], F16, tag="Ot")

            def xs(s, c0, c1):
                return X[:, s * W2 + c0:s * W2 + c1]

            # --- input stream ---------------------------------------------
            # DMA completion sems lag the last byte by ~2.5us (HBM write
            # receipt under load), so consumption is sem-paced; full 512KB
            # slices keep the queues at line rate, and only the tail slice
            # is split so the final chain after the last sem is short.
            # sync queue:   Crow, X0, X2, X4, X6            (2.05 MB + stores)
            # scalar queue: Id,   X1, X3, X5, X7.k1, X7.k2  (2.08 MB + store)
            nc.sync.dma_start(out=Crow[:, :], in_=cf_d[None, :])
            nc.sync.dma_start(out=xs(0, 0, FL), in_=xv[:, 0, 0:FL])
            nc.scalar.dma_start(out=Id[:, :], in_=id_d[:, :])
            nc.scalar.dma_start(out=xs(0, FL, W2), in_=xv[:, 0, FL:W2])
            nc.sync.dma_start(out=xs(1, 0, W2), in_=xv[:, 1, :])
            nc.scalar.dma_start(out=xs(2, 0, W2), in_=xv[:, 2, :])
            nc.sync.dma_start(out=xs(3, 0, W2), in_=xv[:, 3, :])
            nc.scalar.dma_start(out=xs(4, 0, W2), in_=xv[:, 4, :])

            # --- coefficient broadcast ------------------------------------
            # ones @ row as 16 back-to-back N=128 matmuls: ~107ns each even
            # at the cold PE clock, and the burst doubles as HAM warm-up.
            # A and B go to separate PSUM tiles so each staging cast waits
            # only on its own 8 matmuls.
            ones = Crow[0:1, W2:W2 + P]
            ABpA = ppool.tile([P, FL], F32, tag="ABpA")
            ABpB = ppool.tile([P, FL], F32, tag="ABpB")
            for j in range(8):
                nc.tensor.matmul(ABpA[:, j * 128:(j + 1) * 128], ones,
                                 Crow[0:1, j * 128:(j + 1) * 128],
                                 start=True, stop=True)
            nc.vector.tensor_copy(out=ABt[:, 0:FL], in_=ABpA[:, :])
            for j in range(8):
                nc.tensor.matmul(ABpB[:, j * 128:(j + 1) * 128], ones,
                                 Crow[0:1, FL + j * 128:FL + (j + 1) * 128],
                                 start=True, stop=True)
            nc.scalar.copy(out=ABt[:, FL:W2], in_=ABpB[:, :])

            # rest of the input stream (emitted after the casts so the B
            # cast sits before the late gens in the ACT queue's FIFO)
            nc.sync.dma_start(out=xs(5, 0, W2), in_=xv[:, 5, :])
            nc.scalar.dma_start(out=xs(6, 0, W2), in_=xv[:, 6, :])
            nc.sync.dma_start(out=xs(7, 0, FL), in_=xv[:, 7, 0:FL])
            nc.scalar.dma_start(out=xs(7, FL, FL + 768), in_=xv[:, 7, FL:FL + 768])
            nc.sync.dma_start(out=xs(7, FL + 768, W2), in_=xv[:, 7, FL + 768:W2])

            # --- per-slice product + overlap-add --------------------------
            def product(s, c0, c1):
                # T[:, sW2+c0 : sW2+c1] = X ⊙ ABt over that column range
                nc.vector.tensor_tensor(out=T[:, s * W2 + c0:s * W2 + c1],
                                        in0=xs(s, c0, c1),
                                        in1=ABt[:, c0:c1], op=ALU.mult)

            def mms(opt, s, k, cs, start, stop):
                # shifted-identity matmuls for overlap chunks cs of half k
                for c in cs:
                    w = Id[:, 3 - c:3 - c + SL]
                    rhs = T[:, s * W2 + k * FL + c * FS:
                            s * W2 + k * FL + (c + 1) * FS]
                    nc.tensor.matmul(opt[0:SL, 0:FS], w, rhs,
                                     start=(start and c == cs[0]),
                                     stop=(stop and c == cs[-1]))

            for s in range(NS):
                opt = qpool.tile([P, 512], F32, tag="ps", name=f"Opt{s}")
                if s == 0:
                    product(s, 0, FL)
                    mms(opt, s, 0, (0, 1, 2, 3), True, False)
                    product(s, FL, W2)
                    mms(opt, s, 1, (0, 1, 2, 3), False, True)
                elif s == 7:
                    # tail slice in three pieces.  The c=3 chunk of the
                    # overlap-add is partition-aligned (block m reads row
                    # m+3-c = m), so the last piece skips the PE entirely:
                    # after the final DMA sem the chain is one 0.3us DVE
                    # product + one DVE add (PSUM + product -> fp16 out).
                    product(s, 0, FL)
                    mms(opt, s, 0, (0, 1, 2, 3), True, False)
                    product(s, FL, FL + 768)
                    mms(opt, s, 1, (0, 1, 2), False, True)
                    product(s, FL + 768, W2)
                else:
                    product(s, 0, W2)
                    mms(opt, s, 0, (0, 1, 2, 3), True, False)
                    mms(opt, s, 1, (0, 1, 2, 3), False, True)
                # PSUM -> SBUF fp16 staging; the last slice folds the c=3
                # product in during the evacuation add on the DVE
                dst = Ot[0:SL, s * FS:(s + 1) * FS]
                if s == 7:
                    nc.vector.tensor_tensor(
                        out=dst, in0=opt[0:SL, 0:FS],
                        in1=T[0:SL, s * W2 + FL + 768:(s + 1) * W2],
                        op=ALU.add)
                else:
                    nc.scalar.copy(out=dst, in_=opt[0:SL, 0:FS])
                # stores: both mid stores ride the scalar ring AFTER X7.k2 in
                # FIFO order so their bytes never delay the input tail; the
                # small final store goes on the idle sync ring
                if s == 2:
                    nc.scalar.dma_start(out=ov[:, 0:3 * FS], in_=Ot[:, 0:3 * FS])
                elif s == 6:
                    nc.scalar.dma_start(out=ov[:, 3 * FS:7 * FS],
                                        in_=Ot[:, 3 * FS:7 * FS])
                elif s == 7:
                    nc.sync.dma_start(out=ov[:, 7 * FS:8 * FS],
                                      in_=Ot[:, 7 * FS:8 * FS])
    nc.compile()
    return nc


_cache = {}


def _get_nc():
    if "nc" not in _cache:
        _cache["nc"] = build_nc()
    return _cache["nc"]


def make_in_maps(S_real, S_imag):
    a32, b32 = _coeffs32()
    coefs = np.zeros(W2 + P, dtype=np.float16)
    coefs[0:FL] = (a32 * SCALE).astype(np.float16)
    coefs[FL:W2] = (b32 * SCALE).astype(np.float16)
    coefs[W2:] = np.float16(1.0)
    ident = np.eye(P, dtype=ml_dtypes.bfloat16)

    # interleaved + padded fp16 input: row r of core m = global frame
    # m*1024 - 3 + r (zeros outside [0, NF))
    sr16 = S_real.astype(np.float16)
    si16 = S_imag.astype(np.float16)
    glob = np.zeros((3 + NF + P, W2), dtype=np.float16)
    glob[3:3 + NF, 0:FL] = sr16
    glob[3:3 + NF, FL:W2] = si16

    in_maps = []
    for m in range(NCORES):
        base = m * FPC
        x = np.empty((P, NS, W2), dtype=np.float16)
        for s in range(NS):
            x[:, s, :] = glob[base + s * SL:base + s * SL + P]
        in_maps.append({
            "x": x.reshape(-1),
            "coefs": coefs,
            "ident": ident,
        })
    return in_maps


def assemble_output(S_real, S_imag, segs):
    a32, b32 = _coeffs32()
    full = np.empty(OUT_LEN, dtype=np.float32)
    inv_scale = np.float32(1.0) / SCALE
    for m in range(NCORES):
        # seg[p, s*256+r] -> block s*125+p (1000 blocks from the device)
        v = segs[m].reshape(P, NS, FS)[0:SL].transpose(1, 0, 2).reshape(-1)
        bt = m * FPC * FS
        full[bt:bt + NS * SL * FS] = v.astype(np.float32) * inv_scale
        # the core's last 24 blocks: exact f32 on the host
        f0 = m * FPC + 997
        Fr = S_real[f0:f0 + 27] * a32[None, :] + S_imag[f0:f0 + 27] * b32[None, :]
        for lb in range(NS * SL, FPC):
            acc = Fr[lb - 997, 0:FS].copy()
            for c in range(1, 4):
                acc += Fr[lb - c - 997, c * FS:(c + 1) * FS]
            full[(m * FPC + lb) * FS:(m * FPC + lb + 1) * FS] = acc

    # exact f32 recompute of the 6 edge blocks (corr < 2 there: the final
    # division amplifies fp16 error by up to ~1e5) and the global tail
    Fh = S_real[0:3] * a32[None, :] + S_imag[0:3] * b32[None, :]
    full[0:FS] = Fh[0, 0:FS]
    full[FS:2 * FS] = Fh[0, FS:2 * FS] + Fh[1, 0:FS]
    full[2 * FS:3 * FS] = Fh[0, 2 * FS:3 * FS] + Fh[1, FS:2 * FS] + Fh[2, 0:FS]
    Ft = S_real[NF - 3:] * a32[None, :] + S_imag[NF - 3:] * b32[None, :]
    full[NF * FS:NF * FS + FS] = Ft[0, 3 * FS:] + Ft[1, 2 * FS:3 * FS] + Ft[2, FS:2 * FS]
    full[NF * FS + FS:NF * FS + 2 * FS] = Ft[1, 3 * FS:] + Ft[2, 2 * FS:3 * FS]
    full[NF * FS + 2 * FS:] = Ft[2, 3 * FS:]

    if "corr" not in _cache:
        _cache["corr"] = np.maximum(_window_correction(), np.float32(1e-8))
    return full / _cache["corr"]


def kernel(S_real, S_imag):
    S_real = np.asarray(S_real, dtype=np.float32)
    S_imag = np.asarray(S_imag, dtype=np.float32)
    in_maps = make_in_maps(S_real, S_imag)
    nc = _get_nc()
    res = run_bass_kernel_spmd(nc, in_maps, list(range(NCORES)))
    segs = [res.results[m]["out_seg"] for m in range(NCORES)]
    return assemble_output(S_real, S_imag, segs)
